# revision 1
# baseline (speedup 1.0000x reference)
"""Trainium2 Bass kernel for multi-head self-attention (nn_CrossAttention).

Reference computation (B=2, S=4096, C=512, H=8 heads, Dh=64):
    q = hid @ Wq.T; k = hid @ Wk.T; v = hid @ Wv.T     (per-head split)
    out = softmax(q k^T / sqrt(Dh)) v                   (per head)
    final = concat_heads(out) @ Wo.T + bo

Sharding: batch*head parallel. 16 (batch, head) units over 8 cores ->
each core owns one batch b and two adjacent heads. Each core computes a
*partial* output projection (its two heads' contribution to final[b]);
the host sums 4 partials per batch and adds the bias.

Device-side layout choices (see comments inline):
  - hidden is passed pre-transposed (hidT [C, S]) so the model dim (the
    contraction dim of all projections) lies on SBUF partitions.
  - q, k are kept transposed on-chip: qT/kT [128=2*Dh, S].
  - scores are computed transposed (sT [kv, q]) so the PV matmul needs no
    transposition of the 16M-element probability matrix; softmax needs no
    max-pass (scores are O(1) by construction) and the denominator comes
    free from an appended ones-column in V.
  - all matmuls run as float32r (FP22 truncation, full PE rate).
"""

import numpy as np

import concourse.bacc as bacc
import concourse.bass as bass
import concourse.tile as tile
from concourse import mybir
from concourse.bass_utils import run_bass_kernel_spmd

B, S, C = 2, 4096, 512
H, DH = 8, 64
HL = 2                # heads per core
DL = HL * DH          # 128, local projection width
N_CORES = 8
CC = C // 128         # 4 contraction chunks for projections
NQ = S // 512         # 8 q-chunks of 512
NJ = S // 128         # 32 kv-chunks of 128
QG = 1024             # exp granularity along q
NQG = S // QG

F32 = mybir.dt.float32
F32R = mybir.dt.float32r
EXP = mybir.ActivationFunctionType.Exp
RECIP = mybir.ActivationFunctionType.Reciprocal


def _emit(tc, nc, hidT, wqT, wkT, wvT, woT, outp, reps=1, exp_w=None, phases='ABC'):
    with tc.tile_pool(name="persist", bufs=1) as persist:
        qT = persist.tile([DL, S], F32R)
        kT = persist.tile([DL, S], F32R)
        v0 = persist.tile([128, NJ, DH + 1], F32R)   # V plus ones col, head 0
        v1 = persist.tile([128, NJ, DH + 1], F32R)   # head 1
        wo0 = persist.tile([DH, C], F32R)
        wo1 = persist.tile([DH, C], F32R)
        oT0 = persist.tile([DH + 1, S], F32)        # out^T accum + rowsum row
        oT1 = persist.tile([DH + 1, S], F32)

        # whole-tile memset to 1.0: data cols 0..63 are overwritten by the V
        # projection copies below; col 64 stays 1.0 (the rowsum ones-column)
        nc.gpsimd.memset(v0[:].bitcast(F32), 1.0)
        nc.gpsimd.memset(v1[:].bitcast(F32), 1.0)
        nc.gpsimd.memset(oT0[:], 0.0)
        nc.gpsimd.memset(oT1[:], 0.0)

        nc.sync.dma_start(out=wo0[:], in_=woT[0:DH, :])
        nc.sync.dma_start(out=wo1[:], in_=woT[DH:DL, :])

        # ---- phase A: load hidT + weights, project q/k (transposed) and v ----
        if 'A' in phases:
         with tc.tile_pool(name="hload", bufs=1) as hload, \
              tc.tile_pool(name="wload", bufs=1) as wload, \
              tc.tile_pool(name="pjq", bufs=4, space="PSUM") as pjq, \
              tc.tile_pool(name="pjv", bufs=4, space="PSUM") as pjv:
             hid_sb = hload.tile([128, CC, S], F32R)
             hidT_r = hidT.rearrange("(cc p) s -> p cc s", p=128)
             for cc in range(CC):
                 nc.sync.dma_start(out=hid_sb[:, cc, :], in_=hidT_r[:, cc, :])

             wq_sb = wload.tile([128, CC, DL], F32R)
             wk_sb = wload.tile([128, CC, DL], F32R)
             wv_sb = wload.tile([128, CC, DL], F32R)
             for w_sb, w_dram in ((wq_sb, wqT), (wk_sb, wkT), (wv_sb, wvT)):
                 nc.sync.dma_start(
                     out=w_sb[:], in_=w_dram.rearrange("(cc p) d -> p cc d", p=128)
                 )

             # qT/kT: psum[m,n] = sum_c W[m,c] hid[n,c] = qT[dl, s]
             for dst, w_sb in ((qT, wq_sb), (kT, wk_sb)):
                 for sc in range(NQ):
                     ps = pjq.tile([DL, 512], F32)
                     for cc in range(CC):
                         nc.tensor.matmul(
                             ps[:],
                             lhsT=w_sb[:, cc, :],
                             rhs=hid_sb[:, cc, sc * 512:(sc + 1) * 512],
                             start=(cc == 0),
                             stop=(cc == CC - 1),
                         )
                     nc.vector.tensor_copy(dst[:, sc * 512:(sc + 1) * 512], ps[:].bitcast(F32R))

             # v natural: psum[m,n] = sum_c hid[m,c] Wv[n,c] = v[s, dl]
             for jc in range(NJ):
                 ps = pjv.tile([128, DL], F32)
                 for cc in range(CC):
                     nc.tensor.matmul(
                         ps[:],
                         lhsT=hid_sb[:, cc, jc * 128:(jc + 1) * 128],
                         rhs=wv_sb[:, cc, :],
                         start=(cc == 0),
                         stop=(cc == CC - 1),
                     )
                 nc.vector.tensor_copy(v0[:, jc, 0:DH], ps[:, 0:DH].bitcast(F32R))
                 nc.vector.tensor_copy(v1[:, jc, 0:DH], ps[:, DH:DL].bitcast(F32R))

        # ---- phases B+C: attention (q-chunk outer, kv inner) with the
        # normalize + output-projection tail of q-chunk i interleaved into
        # q-chunk i+1's kv loop so it hides under the ACT-bound steady state.
        # PSUM budget (8 banks): st [128,1024] x2 (4) + two [65,512] PV
        # accumulators (2) + out-proj psum [128,512] x2 (2).
        if 'B' not in phases:
            return
        oTn0 = persist.tile([DH, S], F32R)          # normalized out^T
        oTn1 = persist.tile([DH, S], F32R)
        with tc.tile_pool(name="scps", bufs=2, space="PSUM") as scps, \
             tc.tile_pool(name="pvps", bufs=1, space="PSUM") as pvps, \
             tc.tile_pool(name="ptsb", bufs=4) as ptsb, \
             tc.tile_pool(name="norm", bufs=2) as norm, \
             tc.tile_pool(name="ndram", bufs=2, space="DRAM") as ndram, \
             tc.tile_pool(name="opps", bufs=2, space="PSUM") as opps, \
             tc.tile_pool(name="otsb", bufs=2) as otsb:

            def emit_norm(qc):
                # per-head: reciprocal of the rowsum slice, partition-reshaped
                # (via DRAM, which is flat) so the slow DVE divide runs on all
                # 128 lanes; then broadcast back and scale outT
                qo = qc * 512
                for h, (oT, oTn) in enumerate(((oT0, oTn0), (oT1, oTn1))):
                    srow = ndram.tile([1, 512], F32, name="srow", tag="sr")
                    nc.sync.dma_start(out=srow[:], in_=oT[DH:DH + 1, qo:qo + 512])
                    rs = norm.tile([128, 4], F32, name="rs", tag="rs")
                    nc.sync.dma_start(
                        out=rs[:], in_=srow[0, :].rearrange("(p f) -> p f", p=128))
                    nc.vector.reciprocal(rs[:], rs[:])
                    rrow = ndram.tile([1, 512], F32, name="rrow", tag="rr")
                    nc.sync.dma_start(
                        out=rrow[0, :].rearrange("(p f) -> p f", p=128), in_=rs[:])
                    rb = norm.tile([DH, 512], F32, name="rb", tag="rb")
                    r0 = rrow[0, :]
                    bcast = bass.AP(tensor=r0.tensor, offset=r0.offset,
                                    ap=[[0, DH]] + list(r0.ap))
                    nc.sync.dma_start(out=rb[:], in_=bcast)
                    nc.vector.tensor_mul(oTn[:, qo:qo + 512],
                                         oT[0:DH, qo:qo + 512].bitcast(F32R),
                                         rb[:].bitcast(F32R))

            def emit_oproj(sc):
                po = opps.tile([128, C], F32, name="po", tag="po")
                nc.tensor.matmul(po[:], lhsT=oTn0[:, sc * 128:(sc + 1) * 128],
                                 rhs=wo0[:], start=True, stop=False)
                nc.tensor.matmul(po[:], lhsT=oTn1[:, sc * 128:(sc + 1) * 128],
                                 rhs=wo1[:], start=False, stop=True)
                ot = otsb.tile([128, C], F32, name="ot", tag="ot")
                nc.vector.tensor_copy(ot[:], po[:])
                nc.sync.dma_start(out=outp[sc * 128:(sc + 1) * 128, :], in_=ot[:])

            # deferred C-tail work, interleaved into the NEXT q-chunk's kv loop
            pending = []

            def pop_pending():
                if pending:
                    pending.pop(0)()

            for qc in [q for _ in range(reps) for q in range(NQ)]:
                qo = qc * 512
                pva = [pvps.tile([DH + 1, 512], F32, name=f"pvacc{h}",
                                 tag=f"pv{h}") for h in range(HL)]

                def emit_scores(jc):
                    # one [128, 1024] tile = h0's 512 q-cols | h1's 512 q-cols;
                    # the two K=64 matmuls sit in different PE row-groups and
                    # different PSUM banks -> run concurrently
                    st = scps.tile([128, 1024], F32, name="st", tag="st")
                    for h in range(HL):
                        hp = h * DH
                        nc.tensor.matmul(
                            st[:, h * 512:(h + 1) * 512],
                            lhsT=kT[hp:hp + DH, jc * 128:(jc + 1) * 128],
                            rhs=qT[hp:hp + DH, qo:qo + 512],
                            start=True,
                            stop=True,
                        )
                    return st

                # software pipeline: scores(jc+1) are emitted BEFORE pv(jc)
                # so the PE never sits behind the exp in program order
                sts = {0: emit_scores(0)}
                for jc in range(NJ):
                    if jc + 1 < NJ:
                        sts[jc + 1] = emit_scores(jc + 1)
                    st = sts.pop(jc)
                    # exp(score/8) for both heads in ONE ACT instruction;
                    # no max pass (scores are O(1))
                    pt = ptsb.tile([128, 1024], F32R, name="pt", tag="pt")
                    ew = exp_w or 1024
                    nc.scalar.activation(pt[:, 0:ew], st[:, 0:ew], EXP,
                                         scale=0.125)
                    for h, vh in enumerate((v0, v1)):
                        nc.tensor.matmul(
                            pva[h][:],
                            lhsT=vh[:, jc, :],
                            rhs=pt[:, h * 512:(h + 1) * 512],
                            start=(jc == 0),
                            stop=(jc == NJ - 1),
                        )
                    if jc in (2, 8, 14, 20, 26):
                        pop_pending()
                for h, oT in enumerate((oT0, oT1)):
                    nc.vector.tensor_copy(oT[:, qo:qo + 512], pva[h][:])

                if 'C' in phases:
                    pending.append(lambda q=qc: emit_norm(q))
                    for s in range(4 * qc, 4 * qc + 4):
                        pending.append(lambda s=s: emit_oproj(s))

            while pending:
                pop_pending()


def build_nc(reps=1, exp_w=None, phases='ABC'):
    nc = bacc.Bacc("TRN2", target_bir_lowering=False, debug=False)
    hidT = nc.dram_tensor("hidT", [C, S], F32R, kind="ExternalInput").ap()
    wqT = nc.dram_tensor("wqT", [C, DL], F32R, kind="ExternalInput").ap()
    wkT = nc.dram_tensor("wkT", [C, DL], F32R, kind="ExternalInput").ap()
    wvT = nc.dram_tensor("wvT", [C, DL], F32R, kind="ExternalInput").ap()
    woT = nc.dram_tensor("woT", [DL, C], F32R, kind="ExternalInput").ap()
    outp = nc.dram_tensor("outp", [S, C], F32, kind="ExternalOutput").ap()
    with tile.TileContext(nc) as tc:
        _emit(tc, nc, hidT, wqT, wkT, wvT, woT, outp, reps=reps, exp_w=exp_w, phases=phases)
    nc.compile()
    return nc


def make_in_maps(hidden_states, Wq, Wk, Wv, Wo):
    """Shard the full inputs into 8 per-core input maps."""
    hs = np.asarray(hidden_states, dtype=np.float32)
    hidT_b = [np.ascontiguousarray(hs[b].T) for b in range(B)]
    in_maps = []
    for core in range(N_CORES):
        b = core // 4
        p = core % 4
        lo, hi = 2 * p * DH, (2 * p + 2) * DH
        in_maps.append({
            "hidT": hidT_b[b],
            "wqT": np.ascontiguousarray(np.asarray(Wq, np.float32)[lo:hi, :].T),
            "wkT": np.ascontiguousarray(np.asarray(Wk, np.float32)[lo:hi, :].T),
            "wvT": np.ascontiguousarray(np.asarray(Wv, np.float32)[lo:hi, :].T),
            "woT": np.ascontiguousarray(np.asarray(Wo, np.float32)[:, lo:hi].T),
        })
    return in_maps


def gather_output(results, bo):
    """Sum the 4 per-core partial projections per batch, add bias."""
    bo = np.asarray(bo, np.float32)
    out = np.empty((B, S, C), np.float32)
    for b in range(B):
        acc = results[4 * b]["outp"].astype(np.float32).copy()
        for p in range(1, 4):
            acc += results[4 * b + p]["outp"]
        out[b] = acc + bo
    return out


_NC_CACHE = None


def _get_nc():
    global _NC_CACHE
    if _NC_CACHE is None:
        _NC_CACHE = build_nc()
    return _NC_CACHE


def kernel(hidden_states, Wq, Wk, Wv, Wo, bo, _trace=False, _res_out=None):
    nc = _get_nc()
    in_maps = make_in_maps(hidden_states, Wq, Wk, Wv, Wo)
    res = run_bass_kernel_spmd(nc, in_maps, list(range(N_CORES)), trace=_trace)
    if _res_out is not None:
        _res_out.append(res)
    return gather_output(res.results, bo)

